# revision 1
# baseline (speedup 1.0000x reference)
"""Trainium2 Bass kernel for nn_MultiHeadAttention (B=2, S=2048, D=1024, H=16).

Sharding: 8 cores; core c handles batch b=c//4 and the 4 heads
h in [4*(c%4), 4*(c%4)+4). Attention is embarrassingly parallel over (B, H);
the output projection is computed per-core over its head group (partial sums),
and the host sums the 4 partials per batch and adds the output bias.

Matmuls run as float32r (tf32-class: both operands rounded to ~10 mantissa
bits, single-pass, 4x the fp32 streaming rate; measured end-to-end rel err
~5e-4 vs the fp32 reference). Set FAST_MM=False for full-fp32 matmuls
(~2.3e-5, ~3x slower).

Per-core dataflow (contraction dim always on SBUF partitions):
  - host pre-transposes q/k/v per batch -> qT/kT/vT [D, S] (layout prep only)
  - qh^T / kh^T [d, s] computed 2-heads-packed: head A on partitions 0-63,
    head B on 64-127 (lhsT = packed Wq columns, rhs = streamed xT chunks)
  - vh computed in natural [s, d] layout (lhsT = vT chunk, rhs = Wv columns),
    with a ones-column appended -> AV matmul also yields softmax denominators
  - scores computed transposed s^T[k, q] so the softmax numerator
    exp(0.125*s + log2*causal) is produced by ScalarE directly in the
    AV-ready layout (k on partitions); no transposes needed anywhere on-chip.
    The reference's "mask" log(tril*1e-9 + 1e-9) is, by softmax shift
    invariance, exactly a x2 weight on the lower triangle.
  - the two heads of a pack issue score matmuls from partition bases 0/64,
    which the PE runs concurrently (row tiling)
  - AV: psum[65, 512] accumulates vh_aug.T @ e^T over 16 k-chunks; row 64 is
    the denominator. Normalize via DVE reciprocal + partition-broadcast
    (SBUF->SBUF DMA mid-kernel, K=1 PE matmul on the tail).
  - out projection: head pairs stacked to K=128 (odd head hopped to
    partitions 64-127 over a SBUF->SBUF DMA); per (s-block, D-chunk) the two
    pair matmuls accumulate in psum; partial [S, D] DMAed out.
  - attention(qc=0) runs with the k/v projections interleaved at kb
    granularity so the PE computes through the front 16MB k/v DMA window;
    q-projection chunks are emitted just-in-time one qc ahead.
"""
import numpy as np
from contextlib import ExitStack

import concourse.bacc as bacc
import concourse.mybir as mybir
import concourse.tile as tile
from concourse.bass_utils import run_bass_kernel_spmd

F32 = mybir.dt.float32
AF = mybir.ActivationFunctionType
ALU = mybir.AluOpType

B, S, D, H, PD = 2, 2048, 1024, 16, 64
NCORES = 8
HPC = H * B // NCORES        # 4 heads per core
NPACK = HPC // 2             # 2 head-pairs per core
SC = 512                     # free-dim chunk (one fp32 psum bank)
NSC = S // SC                # 4
NKB = S // 128               # 16 key blocks / s blocks
NDC = D // 128               # 8 contraction chunks for the projections
LOG2 = float(np.log(2.0))

# cst blob column layout (per partition); small always-needed scalars first,
# the 1MB of diagonal masks last so their DMA can trail the first data chunks
CST_BQ = 0                   # [2] per-pack bq (per-partition scalars)
CST_BK = CST_BQ + 2          # [2]
CST_BV = CST_BK + 2          # [256] bv broadcast (free-dim layout)
CST_LOG2 = CST_BV + HPC * PD # [1] log(2) per partition (exp bias)
CST_ZERO = CST_LOG2 + 1      # [1] 0.0 per partition (exp bias)
CST_ONE = CST_ZERO + 1       # [1] 1.0 per partition
CST_MASK = CST_ONE + 1       # [4, 512] diagonal-block additive log-masks
CST_COLS = CST_MASK + 4 * SC


def _build(causal: bool, fast_mm: bool = False):
    """fast_mm: run matmuls as float32r (single-pass, 4x streaming rate,
    reduced product precision) instead of full fp32."""
    nc = bacc.Bacc()
    MMD = mybir.dt.float32r if fast_mm else F32
    qT = nc.dram_tensor("qT", [D, S], MMD, kind="ExternalInput")
    kT = nc.dram_tensor("kT", [D, S], MMD, kind="ExternalInput")
    vT = nc.dram_tensor("vT", [D, S], MMD, kind="ExternalInput")
    wq = nc.dram_tensor("wq", [D, HPC * PD], MMD, kind="ExternalInput")
    wk = nc.dram_tensor("wk", [D, HPC * PD], MMD, kind="ExternalInput")
    wv = nc.dram_tensor("wv", [D, HPC * PD], MMD, kind="ExternalInput")
    wo = nc.dram_tensor("wo", [HPC * PD, D], MMD, kind="ExternalInput")
    cst = nc.dram_tensor("cst", [128, CST_COLS], F32, kind="ExternalInput")
    out_d = nc.dram_tensor("out", [S, D], F32, kind="ExternalOutput")

    mm = nc.tensor.matmul

    with tile.TileContext(nc) as tc, ExitStack() as ctx:
        cpool = ctx.enter_context(tc.tile_pool(name="cpool", bufs=1))
        xpool = ctx.enter_context(tc.tile_pool(name="xpool", bufs=2))
        hpool = ctx.enter_context(tc.tile_pool(name="hpool", bufs=1))
        epool = ctx.enter_context(tc.tile_pool(name="epool", bufs=3))
        opool = ctx.enter_context(tc.tile_pool(name="opool", bufs=2))
        spool = ctx.enter_context(tc.tile_pool(name="spool", bufs=2))
        pspool = ctx.enter_context(tc.tile_pool(name="ps", bufs=2, space="PSUM"))

        # ---- constants; HWDGE DMAs drain FIFO in emission order, so emit
        # in first-use order: wq+cst (first q-proj chunk), then k/v weights ----
        wq_t = cpool.tile([128, NDC, HPC * PD], MMD)
        wq_src = wq[:].rearrange("(c p) m -> p c m", p=128)
        nc.sync.dma_start(wq_t[:, 0:NDC // 2, :], wq_src[:, 0:NDC // 2, :])
        nc.sync.dma_start(wq_t[:, NDC // 2:, :], wq_src[:, NDC // 2:, :])
        cst_t = cpool.tile([128, CST_COLS], F32)
        nc.sync.dma_start(cst_t[:, 0:CST_MASK], cst[:, 0:CST_MASK])
        ones1 = cpool.tile([1, PD], F32)
        nc.vector.memset(ones1[:], 1.0)

        def mask_ap(delta):
            return cst_t[:, CST_MASK + delta * SC: CST_MASK + (delta + 1) * SC]

        qh = [hpool.tile([128, S], MMD, name=f"qh{p}") for p in range(NPACK)]
        kh = [hpool.tile([128, S], MMD, name=f"kh{p}") for p in range(NPACK)]
        vh_all = hpool.tile([128, NKB, HPC, PD + 1], MMD, name="vh_all")
        nc.vector.tensor_copy(
            vh_all[:, :, :, PD:PD + 1],
            cst_t[:, CST_ONE:CST_ONE + 1].to_broadcast((128, NKB, HPC, 1)))

        def qk_proj(xdram, wtile, htiles, boff, sc, split_dma=False):
            """One s-chunk of the packed ^T projection for q or k."""
            xTc = xpool.tile([128, NDC, SC], MMD, tag="xTc", name="xTc",
                             bufs=3)
            xsrc = xdram[:, sc * SC:(sc + 1) * SC].rearrange(
                "(c p) s -> p c s", p=128)
            if split_dma:
                q4 = NDC // 4
                for i in range(4):
                    nc.sync.dma_start(xTc[:, i * q4:(i + 1) * q4, :],
                                      xsrc[:, i * q4:(i + 1) * q4, :])
            else:
                nc.sync.dma_start(xTc[:], xsrc)
            for pk in range(NPACK):
                ps = pspool.tile([128, SC], F32, tag="mm", name="ps_qk")
                for dc in range(NDC):
                    mm(ps[:],
                       wtile[:, dc, pk * 128:(pk + 1) * 128],
                       xTc[:, dc, :],
                       start=(dc == 0), stop=(dc == NDC - 1))
                nc.vector.tensor_scalar(
                    htiles[pk][:, sc * SC:(sc + 1) * SC], ps[:],
                    cst_t[:, boff + pk: boff + pk + 1], None, ALU.add)

        bv_ap = cst_t[:, CST_BV: CST_BV + HPC * PD].rearrange(
            "p (h d) -> p h d", h=HPC)

        def v_proj(sb):
            """One 128-row block of the natural-layout v projection."""
            vsl = xpool.tile([128, NDC, 128], MMD, tag="vsl", name="vsl")
            vsrc = vT[:, sb * 128:(sb + 1) * 128].rearrange(
                "(c p) j -> p c j", p=128)
            nc.sync.dma_start(vsl[:, 0:NDC // 2, :], vsrc[:, 0:NDC // 2, :])
            nc.sync.dma_start(vsl[:, NDC // 2:, :], vsrc[:, NDC // 2:, :])
            ps = pspool.tile([128, HPC * PD], F32, tag="mm", name="ps_v")
            for dc in range(NDC):
                mm(ps[:], vsl[:, dc, :], wv_t[:, dc, :],
                   start=(dc == 0), stop=(dc == NDC - 1))
            nc.vector.tensor_tensor(
                vh_all[:, sb, :, 0:PD],
                ps[:].rearrange("p (h d) -> p h d", h=HPC),
                bv_ap,
                ALU.add)

        def score_exp_pair(qc, pk, hh, pair):
            """Scores^T for TWO consecutive k-blocks of one head into one
            2-bank psum tile, then a single [128, 2*SC] exp -> et2.

            Halves the ScalarE instruction count (its per-op PSUM-access
            overhead made the dense phases ACT-bound)."""
            base = hh * PD
            sps = pspool.tile([128, 2, SC], F32, tag=f"s2h{hh}", name="sps",
                              bufs=1)
            for j in range(2):
                kb = 2 * pair + j
                mm(sps[:, j, :],
                   kh[pk][base:base + PD, kb * 128:(kb + 1) * 128],
                   qh[pk][base:base + PD, qc * SC:(qc + 1) * SC])
            et2 = epool.tile([128, 2, SC], MMD, tag=f"e{hh}", name=f"et{hh}",
                             bufs=3)
            kb0 = 2 * pair
            delta = kb0 - 4 * qc
            if causal and 0 <= delta < 4:
                # both k-blocks of the pair are diagonal blocks (4qc is even),
                # and their two mask tiles are adjacent cst columns
                tmp = epool.tile([128, 2, SC], F32, tag="tmp", name="tmp",
                                 bufs=2)
                moff = CST_MASK + delta * SC
                nc.vector.scalar_tensor_tensor(
                    tmp[:], sps[:], 0.125,
                    cst_t[:, moff:moff + 2 * SC].rearrange(
                        "p (j s) -> p j s", j=2),
                    ALU.mult, ALU.add)
                nc.scalar.activation(
                    et2[:], tmp[:], AF.Exp,
                    bias=cst_t[:, CST_ZERO:CST_ZERO + 1], scale=1.0)
            else:
                boff = CST_LOG2 if (causal and delta < 0) else CST_ZERO
                nc.scalar.activation(
                    et2[:], sps[:], AF.Exp,
                    bias=cst_t[:, boff:boff + 1], scale=0.125)
            return et2

        def av_mm(av, i4, kb, et):
            mm(av[:], vh_all[:, kb, i4, :], et[:],
               start=(kb == 0), stop=(kb == NKB - 1))

        def attention_pack(qc, pk, with_kv_proj=False, tasks=None):
            """8 k-block-pair sweep for one pack (2 heads), AV one pair behind.
            Returns the pack's two [65, SC] psum accumulators."""
            avs = [pspool.tile([PD + 1, SC], F32, tag="av", name=f"av{hh}",
                               bufs=2)
                   for hh in range(2)]
            prevs = None
            npair = NKB // 2
            for pair in range(npair):
                if with_kv_proj:
                    if pair % 2 == 0:
                        qk_proj(kT, wk_t, kh, CST_BK, pair // 2,
                                split_dma=True)
                    v_proj(2 * pair)
                if tasks is not None:
                    for t in tasks.get(pair, ()):
                        t()
                cur = [score_exp_pair(qc, pk, hh, pair) for hh in range(2)]
                if with_kv_proj:
                    v_proj(2 * pair + 1)
                if prevs is not None:
                    for hh in range(2):
                        for j in range(2):
                            av_mm(avs[hh], pk * 2 + hh, 2 * (pair - 1) + j,
                                  prevs[hh][:, j, :])
                prevs = cur
            for hh in range(2):
                for j in range(2):
                    av_mm(avs[hh], pk * 2 + hh, NKB - 2 + j,
                          prevs[hh][:, j, :])
            return avs

        def normalize_pack(avs, pk, ohs, low_latency=False):
            """outh^T = av[0:64] * bcast(1/av[64]) for the pack's 2 heads,
            stacked onto one [128, SC] tile (odd head hops to partitions
            64-127 over a SBUF->SBUF DMA) so the out-projection runs K=128."""
            oh = opool.tile([128, SC], MMD, tag=f"ohp{pk}", name=f"ohp{pk}")
            ohs.append(oh)
            for hh in (1, 0):  # odd first: its stack DMA overlaps hh=0's chain
                av = avs[hh]
                rrow = spool.tile([1, SC], F32, tag="rrow", name="rrow")
                nc.vector.reciprocal(rrow[:], av[PD:PD + 1, :])
                if low_latency:
                    bps = pspool.tile([PD, SC], F32, tag="mm", name="bps")
                    nc.tensor.matmul(bps[:], ones1[:], rrow[:])
                    rb = spool.tile([PD, SC], F32, tag="rb", name="rb")
                    nc.vector.tensor_copy(rb[:], bps[:])
                else:
                    rb = spool.tile([PD, SC], F32, tag="rb", name="rb")
                    nc.sync.dma_start(
                        rb[:],
                        rrow[0:1, :].rearrange("p (o s) -> p o s",
                                               o=1).broadcast_to((1, PD, SC)))
                if hh == 0:
                    nc.vector.tensor_tensor(oh[0:PD, :], av[0:PD, :], rb[:],
                                            ALU.mult)
                else:
                    stg = spool.tile([PD, SC], MMD, tag="stg", name="stg")
                    nc.vector.tensor_tensor(stg[:], av[0:PD, :], rb[:],
                                            ALU.mult)
                    nc.sync.dma_start(oh[PD:128, :], stg[:])

        def proj_out_group(qc, ohs, sbl, dc2):
            sb = qc * 4 + sbl
            pps = pspool.tile([128, SC], F32, tag="mm", name="pps")
            for pr in range(NPACK):
                mm(pps[:],
                   ohs[pr][:, sbl * 128:(sbl + 1) * 128],
                   wo_t[:, pr, dc2 * SC:(dc2 + 1) * SC],
                   start=(pr == 0), stop=(pr == NPACK - 1))
            oev = opool.tile([128, SC], F32, tag="oev", name="oev",
                             bufs=3)
            nc.vector.tensor_copy(oev[:], pps[:])
            nc.sync.dma_start(
                out_d[sb * 128:(sb + 1) * 128,
                      dc2 * SC:(dc2 + 1) * SC],
                oev[:])

        # ---- phase 1: qh chunk 0, then attention(qc=0, both packs) with the
        # k/v projections interleaved at kb granularity ----
        qk_proj(qT, wq_t, qh, CST_BQ, 0, split_dma=True)
        half = CST_MASK + 2 * SC
        nc.sync.dma_start(cst_t[:, CST_MASK:half], cst[:, CST_MASK:half])
        nc.sync.dma_start(cst_t[:, half:CST_COLS], cst[:, half:CST_COLS])
        wk_t = cpool.tile([128, NDC, HPC * PD], MMD)
        wk_src = wk[:].rearrange("(c p) m -> p c m", p=128)
        nc.sync.dma_start(wk_t[:, 0:NDC // 2, :], wk_src[:, 0:NDC // 2, :])
        nc.sync.dma_start(wk_t[:, NDC // 2:, :], wk_src[:, NDC // 2:, :])
        wv_t = cpool.tile([128, NDC, HPC * PD], MMD)
        wv_src = wv[:].rearrange("(c p) m -> p c m", p=128)
        nc.sync.dma_start(wv_t[:, 0:NDC // 2, :], wv_src[:, 0:NDC // 2, :])
        nc.sync.dma_start(wv_t[:, NDC // 2:, :], wv_src[:, NDC // 2:, :])
        avs0 = attention_pack(0, 0, with_kv_proj=True)

        # wo arrives while attention runs; needed first at proj_out(0)
        wo_t = cpool.tile([128, NPACK, D], MMD)
        nc.sync.dma_start(wo_t[:], wo[:].rearrange("(r p) n -> p r n", p=128))

        ohs = []
        normalize_pack(avs0, 0, ohs)
        avs1 = attention_pack(0, 1)
        qk_proj(qT, wq_t, qh, CST_BQ, 1, split_dma=True)
        normalize_pack(avs1, 1, ohs)

        prev_ohs = ohs
        for qc in range(1, NSC):
            # while this qc's (ScalarE-bound) sweep runs, fill the PE's slack
            # with the previous qc's out-projection and the next qc's
            # q-projection chunk instead of serializing them at the boundary
            tasks0, tasks1 = {}, {}
            po, pq = prev_ohs, qc - 1
            for g in range(8):
                tasks0.setdefault(g, []).append(
                    (lambda s=g // 2, d=g % 2, o=po, q=pq:
                     proj_out_group(q, o, s, d)))
            if qc < NSC - 1:
                tasks1.setdefault(2, []).append(
                    lambda s=qc + 1: qk_proj(qT, wq_t, qh, CST_BQ, s,
                                             split_dma=True))
            ohs = []
            avs0 = attention_pack(qc, 0, tasks=tasks0)
            normalize_pack(avs0, 0, ohs, low_latency=(qc == NSC - 1))
            avs1 = attention_pack(qc, 1, tasks=tasks1)
            normalize_pack(avs1, 1, ohs, low_latency=(qc == NSC - 1))
            prev_ohs = ohs
        for sbl in range(4):
            for dc2 in range(2):
                proj_out_group(NSC - 1, ohs, sbl, dc2)

    nc.compile()
    return nc


_programs = {}


FAST_MM = True


def _get_program(causal: bool):
    key = (causal, FAST_MM)
    if key not in _programs:
        _programs[key] = _build(causal, FAST_MM)
    return _programs[key]


def _make_cst(bq4, bk4, bv4, causal: bool) -> np.ndarray:
    """Per-core constant blob [128, CST_COLS]."""
    cst = np.zeros((128, CST_COLS), np.float32)
    # diagonal-block additive log-masks: log(2) iff
    # q_local - 128*delta >= k_local (else 0); zeros when not causal
    for delta in range(4):
        if causal:
            kloc = np.arange(128)[:, None]
            qloc = np.arange(SC)[None, :]
            m = np.where(qloc - 128 * delta >= kloc, LOG2, 0.0)
        else:
            m = np.zeros((128, SC))
        cst[:, CST_MASK + delta * SC: CST_MASK + (delta + 1) * SC] = m
    # per-pack per-partition biases: partition p of pack pk is d = pk*128+p
    cst[:, CST_BQ:CST_BQ + 2] = bq4.reshape(2, 128).T
    cst[:, CST_BK:CST_BK + 2] = bk4.reshape(2, 128).T
    # bv in free-dim layout [4*64], broadcast along partitions
    cst[:, CST_BV:CST_BV + HPC * PD] = np.broadcast_to(
        bv4, (128, HPC * PD))
    cst[:, CST_LOG2] = LOG2
    cst[:, CST_ZERO] = 0.0
    cst[:, CST_ONE] = 1.0
    return cst


def kernel(**inputs) -> np.ndarray:
    q = np.asarray(inputs["q"], np.float32)
    k = np.asarray(inputs["k"], np.float32)
    v = np.asarray(inputs["v"], np.float32)
    Wq = np.asarray(inputs["Wq"], np.float32)
    Wk = np.asarray(inputs["Wk"], np.float32)
    Wv = np.asarray(inputs["Wv"], np.float32)
    Wo = np.asarray(inputs["Wo"], np.float32)
    bq = np.asarray(inputs["bq"], np.float32)
    bk = np.asarray(inputs["bk"], np.float32)
    bv = np.asarray(inputs["bv"], np.float32)
    bo = np.asarray(inputs["bo"], np.float32)
    causal = bool(np.asarray(inputs["use_causal_mask"]).item())

    nc = _get_program(causal)

    qTb = [np.ascontiguousarray(q[b].T) for b in range(B)]
    kTb = [np.ascontiguousarray(k[b].T) for b in range(B)]
    vTb = [np.ascontiguousarray(v[b].T) for b in range(B)]

    in_maps = []
    for c in range(NCORES):
        b, hg = divmod(c, NCORES // B)
        cols = slice(hg * HPC * PD, (hg + 1) * HPC * PD)
        in_maps.append({
            "qT": qTb[b],
            "kT": kTb[b],
            "vT": vTb[b],
            "wq": np.ascontiguousarray(Wq[:, cols]),
            "wk": np.ascontiguousarray(Wk[:, cols]),
            "wv": np.ascontiguousarray(Wv[:, cols]),
            "wo": np.ascontiguousarray(Wo[cols, :]),
            "cst": _make_cst(bq[cols], bk[cols], bv[cols], causal),
        })

    res = run_bass_kernel_spmd(nc, in_maps, list(range(NCORES)))

    out = np.empty((B, S, D), np.float32)
    ncb = NCORES // B
    for b in range(B):
        acc = res.results[b * ncb]["out"].copy()
        for c in range(b * ncb + 1, (b + 1) * ncb):
            acc += res.results[c]["out"]
        out[b] = acc + bo
    return out



# revision 3
# speedup vs baseline: 1.0747x; 1.0747x over previous
"""Trainium2 Bass kernel for nn_MultiHeadAttention (B=2, S=2048, D=1024, H=16).

Sharding: 8 cores; core c handles batch b=c//4 and the 4 heads
h in [4*(c%4), 4*(c%4)+4). Attention is embarrassingly parallel over (B, H);
the output projection is computed per-core over its head group (partial sums),
and the host sums the 4 partials per batch and adds the output bias.

All matmul operands are fp16 (10 mantissa bits — the same precision class as
tf32/float32r, measured end-to-end rel err ~5e-4) with fp32 PSUM
accumulation. fp16 halves every DMA against the serial 360GB/s DMA-engine
resource and halves SBUF footprints vs fp32r at the same 1 row/cycle PE rate.

Per-core dataflow (contraction dim always on SBUF partitions):
  - host pre-packs q/k/v per batch into the exact per-partition SBUF layouts
    (flat [128, ...] slabs so every DMA descriptor is >=1KB: no
    small-descriptor 2x penalty) and converts to fp16
  - qh^T / kh^T [d, s] computed 2-heads-packed: head A on partitions 0-63,
    head B on 64-127 (lhsT = packed Wq columns, rhs = streamed xT chunks)
  - vh computed in natural [s, d] layout, with a ones-column appended -> the
    AV matmul also yields the softmax denominators
  - scores computed transposed s^T[k, q] so the softmax numerator
    exp(0.125*s + log2*causal) is produced by ScalarE directly in the
    AV-ready layout (k on partitions); no transposes needed anywhere on-chip.
    The reference's "mask" log(tril*1e-9 + 1e-9) is, by softmax shift
    invariance, exactly a x2 weight on the lower triangle.
  - AV: psum[65, 512] accumulates vh_aug.T @ e^T over 16 k-chunks; row 64 is
    the denominator. Normalize via DVE reciprocal + partition-broadcast.
  - out projection: head pairs stacked to K=128 (odd head hopped to
    partitions 64-127 over a SBUF->SBUF DMA); per (s-block, D-chunk) the two
    pair matmuls accumulate in psum; partial [S, D] DMAed out in fp16.
  - schedule: attention sweeps are ACT(exp)-paced (~2076ns/pair vs ~1706ns of
    score+AV PE work), so every projection matmul (k/v at qc=0, q chunk
    prefetch, out-projection groups) is spread as per-pair fill across BOTH
    packs' sweeps to keep the PE saturated.
  - tail: the last q-chunk's out-projection is split by pack: pack0's half
    runs as fill inside pack1's final sweep (into a separate DRAM slab the
    host adds back), so only 8 single-matmul groups remain after the last
    normalize.
"""
import numpy as np
from contextlib import ExitStack

import concourse.bacc as bacc
import concourse.mybir as mybir
import concourse.tile as tile
from concourse.bass_utils import run_bass_kernel_spmd

F32 = mybir.dt.float32
F16 = mybir.dt.float16
AF = mybir.ActivationFunctionType
ALU = mybir.AluOpType

B, S, D, H, PD = 2, 2048, 1024, 16, 64
NCORES = 8
HPC = H * B // NCORES        # 4 heads per core
NPACK = HPC // 2             # 2 head-pairs per core
HPD = HPC * PD               # 256 projected columns per core
SC = 512                     # free-dim chunk (one fp32 psum bank)
NSC = S // SC                # 4
NKB = S // 128               # 16 key blocks / s blocks
NDC = D // 128               # 8 contraction chunks for the projections
LOG2 = float(np.log(2.0))

# fp32 cst blob column layout (per partition): small always-needed scalars
CST_BQ = 0                   # [2] per-pack bq (per-partition scalars)
CST_BK = CST_BQ + 2          # [2]
CST_BV = CST_BK + 2          # [256] bv broadcast (free-dim layout)
CST_LOG2 = CST_BV + HPD      # [1] log(2) per partition (exp bias)
CST_ZERO = CST_LOG2 + 1      # [1] 0.0 per partition (exp bias)
CST_ONE = CST_ZERO + 1       # [1] 1.0 per partition
CST_COLS = CST_ONE + 1


def _build(causal: bool):
    nc = bacc.Bacc()
    qp = nc.dram_tensor("qp", [128, NSC * NDC * SC], F16, kind="ExternalInput")
    kp = nc.dram_tensor("kp", [128, NSC * NDC * SC], F16, kind="ExternalInput")
    vp = nc.dram_tensor("vp", [128, NKB * NDC * 128], F16,
                        kind="ExternalInput")
    wq = nc.dram_tensor("wq", [128, NDC * HPD], F16, kind="ExternalInput")
    wk = nc.dram_tensor("wk", [128, NDC * HPD], F16, kind="ExternalInput")
    wv = nc.dram_tensor("wv", [128, NDC * HPD], F16, kind="ExternalInput")
    wo = nc.dram_tensor("wo", [128, NPACK * D], F16, kind="ExternalInput")
    cst = nc.dram_tensor("cst", [128, CST_COLS], F32, kind="ExternalInput")
    msk = nc.dram_tensor("msk", [128, 4 * SC], F16, kind="ExternalInput")
    out_d = nc.dram_tensor("out", [S, D], F16, kind="ExternalOutput")
    out2_d = nc.dram_tensor("out2", [SC, D], F16, kind="ExternalOutput")

    mm = nc.tensor.matmul

    with tile.TileContext(nc) as tc, ExitStack() as ctx:
        cpool = ctx.enter_context(tc.tile_pool(name="cpool", bufs=1))
        xpool = ctx.enter_context(tc.tile_pool(name="xpool", bufs=2))
        hpool = ctx.enter_context(tc.tile_pool(name="hpool", bufs=1))
        epool = ctx.enter_context(tc.tile_pool(name="epool", bufs=3))
        opool = ctx.enter_context(tc.tile_pool(name="opool", bufs=2))
        spool = ctx.enter_context(tc.tile_pool(name="spool", bufs=2))
        pspool = ctx.enter_context(tc.tile_pool(name="ps", bufs=2, space="PSUM"))

        # ---- constants; HWDGE DMAs drain FIFO in emission order, so emit
        # in first-use order: wq+cst (first q-proj chunk), then k/v weights ----
        wq_t = cpool.tile([128, NDC * HPD], F16)
        nc.sync.dma_start(wq_t[:, 0:NDC * HPD // 2], wq[:, 0:NDC * HPD // 2])
        nc.sync.dma_start(wq_t[:, NDC * HPD // 2:], wq[:, NDC * HPD // 2:])
        cst_t = cpool.tile([128, CST_COLS], F32)
        nc.sync.dma_start(cst_t[:], cst[:])
        ones1 = cpool.tile([1, PD], F16)
        nc.vector.memset(ones1[:], 1.0)
        msk_t = cpool.tile([128, 4 * SC], F16)

        qh = [hpool.tile([128, S], F16, name=f"qh{p}") for p in range(NPACK)]
        kh = [hpool.tile([128, S], F16, name=f"kh{p}") for p in range(NPACK)]
        vh_all = hpool.tile([128, NKB, HPC, PD + 1], F16, name="vh_all")
        nc.vector.tensor_copy(
            vh_all[:, :, :, PD:PD + 1],
            cst_t[:, CST_ONE:CST_ONE + 1].to_broadcast((128, NKB, HPC, 1)))

        def qk_proj(xdram, wtile, htiles, boff, sc, nsplit=4):
            """One s-chunk of the packed ^T projection for q or k."""
            xTc = xpool.tile([128, NDC * SC], F16, tag="xTc", name="xTc",
                             bufs=3)
            w = NDC * SC // nsplit
            for i in range(nsplit):
                nc.sync.dma_start(
                    xTc[:, i * w:(i + 1) * w],
                    xdram[:, sc * NDC * SC + i * w:
                          sc * NDC * SC + (i + 1) * w])
            for pk in range(NPACK):
                ps = pspool.tile([128, SC], F32, tag="mm", name="ps_qk")
                for dc in range(NDC):
                    mm(ps[:],
                       wtile[:, dc * HPD + pk * 128:dc * HPD + (pk + 1) * 128],
                       xTc[:, dc * SC:(dc + 1) * SC],
                       start=(dc == 0), stop=(dc == NDC - 1))
                nc.vector.tensor_scalar(
                    htiles[pk][:, sc * SC:(sc + 1) * SC], ps[:],
                    cst_t[:, boff + pk: boff + pk + 1], None, ALU.add)

        bv_ap = cst_t[:, CST_BV: CST_BV + HPD].rearrange(
            "p (h d) -> p h d", h=HPC)

        def v_proj(sb):
            """One 128-row block of the natural-layout v projection."""
            vsl = xpool.tile([128, NDC * 128], F16, tag="vsl", name="vsl")
            nc.sync.dma_start(
                vsl[:], vp[:, sb * NDC * 128:(sb + 1) * NDC * 128])
            ps = pspool.tile([128, HPD], F32, tag="mm", name="ps_v")
            for dc in range(NDC):
                mm(ps[:], vsl[:, dc * 128:(dc + 1) * 128],
                   wv_t[:, dc * HPD:(dc + 1) * HPD],
                   start=(dc == 0), stop=(dc == NDC - 1))
            nc.vector.tensor_tensor(
                vh_all[:, sb, :, 0:PD],
                ps[:].rearrange("p (h d) -> p h d", h=HPC),
                bv_ap,
                ALU.add)

        def score_exp_pair(qc, pk, hh, pair):
            """Scores^T for TWO consecutive k-blocks of one head into one
            2-bank psum tile, then a single [128, 2*SC] exp -> et2.

            Halves the ScalarE instruction count (its per-op PSUM-access
            overhead made the dense phases ACT-bound)."""
            base = hh * PD
            sps = pspool.tile([128, 2, SC], F32, tag=f"s2h{hh}", name="sps",
                              bufs=1)
            for j in range(2):
                kb = 2 * pair + j
                mm(sps[:, j, :],
                   kh[pk][base:base + PD, kb * 128:(kb + 1) * 128],
                   qh[pk][base:base + PD, qc * SC:(qc + 1) * SC])
            et2 = epool.tile([128, 2, SC], F16, tag=f"e{hh}", name=f"et{hh}",
                             bufs=3)
            kb0 = 2 * pair
            delta = kb0 - 4 * qc
            if causal and 0 <= delta < 4:
                # both k-blocks of the pair are diagonal blocks (4qc is even),
                # and their two mask tiles are adjacent msk columns
                tmp = epool.tile([128, 2, SC], F16, tag="tmp", name="tmp",
                                 bufs=2)
                moff = delta * SC
                nc.vector.scalar_tensor_tensor(
                    tmp[:], sps[:], 0.125,
                    msk_t[:, moff:moff + 2 * SC].rearrange(
                        "p (j s) -> p j s", j=2),
                    ALU.mult, ALU.add)
                nc.scalar.activation(
                    et2[:], tmp[:], AF.Exp,
                    bias=cst_t[:, CST_ZERO:CST_ZERO + 1], scale=1.0)
            else:
                boff = CST_LOG2 if (causal and delta < 0) else CST_ZERO
                nc.scalar.activation(
                    et2[:], sps[:], AF.Exp,
                    bias=cst_t[:, boff:boff + 1], scale=0.125)
            return et2

        def av_mm(av, i4, kb, et):
            mm(av[:], vh_all[:, kb, i4, :], et[:],
               start=(kb == 0), stop=(kb == NKB - 1))

        def attention_pack(qc, pk, with_kv_proj=False, tasks=None):
            """8 k-block-pair sweep for one pack (2 heads), AV one pair behind.
            Returns the pack's two [65, SC] psum accumulators."""
            avs = [pspool.tile([PD + 1, SC], F32, tag="av", name=f"av{hh}",
                               bufs=2)
                   for hh in range(2)]
            prevs = None
            npair = NKB // 2
            for pair in range(npair):
                if with_kv_proj:
                    if pair % 2 == 0:
                        qk_proj(kT_d, wk_t, kh, CST_BK, pair // 2)
                    v_proj(2 * pair)
                if tasks is not None:
                    for t in tasks.get(pair, ()):
                        t()
                cur = [score_exp_pair(qc, pk, hh, pair) for hh in range(2)]
                if with_kv_proj:
                    v_proj(2 * pair + 1)
                if prevs is not None:
                    for hh in range(2):
                        for j in range(2):
                            av_mm(avs[hh], pk * 2 + hh, 2 * (pair - 1) + j,
                                  prevs[hh][:, j, :])
                prevs = cur
            for hh in range(2):
                for j in range(2):
                    av_mm(avs[hh], pk * 2 + hh, NKB - 2 + j,
                          prevs[hh][:, j, :])
            return avs

        def normalize_pack(avs, pk, ohs, low_latency=False):
            """outh^T = av[0:64] * bcast(1/av[64]) for the pack's 2 heads,
            stacked onto one [128, SC] tile (odd head hops to partitions
            64-127 over a SBUF->SBUF DMA) so the out-projection runs K=128."""
            oh = opool.tile([128, SC], F16, tag=f"ohp{pk}", name=f"ohp{pk}")
            ohs.append(oh)
            for hh in (1, 0):  # odd first: its stack DMA overlaps hh=0's chain
                av = avs[hh]
                rrow = spool.tile([1, SC], F16, tag="rrow", name="rrow")
                with nc.allow_low_precision(
                        reason="fp16 1/denominator: 2^-11 rel, within budget"):
                    nc.vector.reciprocal(rrow[:], av[PD:PD + 1, :])
                if low_latency:
                    bps = pspool.tile([PD, SC], F32, tag="mm", name="bps")
                    nc.tensor.matmul(bps[:], ones1[:], rrow[:])
                    rb = spool.tile([PD, SC], F32, tag="rb", name="rb")
                    nc.vector.tensor_copy(rb[:], bps[:])
                else:
                    rb = spool.tile([PD, SC], F16, tag="rb16", name="rb16")
                    nc.sync.dma_start(
                        rb[:],
                        rrow[0:1, :].rearrange("p (o s) -> p o s",
                                               o=1).broadcast_to((1, PD, SC)))
                if hh == 0:
                    nc.vector.tensor_tensor(oh[0:PD, :], av[0:PD, :], rb[:],
                                            ALU.mult)
                else:
                    stg = spool.tile([PD, SC], F16, tag="stg", name="stg")
                    nc.vector.tensor_tensor(stg[:], av[0:PD, :], rb[:],
                                            ALU.mult)
                    nc.sync.dma_start(oh[PD:128, :], stg[:])

        def proj_out_group(qc, ohs, sbl, dc2, prs=(0, 1), dest=None,
                           row_base=None):
            sb = qc * 4 + sbl if row_base is None else row_base + sbl
            dest = out_d if dest is None else dest
            pps = pspool.tile([128, SC], F32, tag="mm", name="pps")
            for i, pr in enumerate(prs):
                mm(pps[:],
                   ohs[pr][:, sbl * 128:(sbl + 1) * 128],
                   wo_t[:, pr * D + dc2 * SC:pr * D + (dc2 + 1) * SC],
                   start=(i == 0), stop=(i == len(prs) - 1))
            oev = opool.tile([128, SC], F16, tag="oev", name="oev",
                             bufs=3)
            nc.vector.tensor_copy(oev[:], pps[:])
            nc.sync.dma_start(
                dest[sb * 128:(sb + 1) * 128,
                     dc2 * SC:(dc2 + 1) * SC],
                oev[:])

        # ---- phase 1: qh chunk 0, then attention(qc=0, both packs) with the
        # k/v projections interleaved at kb granularity ----
        kT_d = kp
        qk_proj(qp, wq_t, qh, CST_BQ, 0)
        if causal:
            nc.sync.dma_start(msk_t[:, 0:2 * SC], msk[:, 0:2 * SC])
            nc.sync.dma_start(msk_t[:, 2 * SC:], msk[:, 2 * SC:])
        wk_t = cpool.tile([128, NDC * HPD], F16)
        nc.sync.dma_start(wk_t[:, 0:NDC * HPD // 2], wk[:, 0:NDC * HPD // 2])
        nc.sync.dma_start(wk_t[:, NDC * HPD // 2:], wk[:, NDC * HPD // 2:])
        wv_t = cpool.tile([128, NDC * HPD], F16)
        nc.sync.dma_start(wv_t[:, 0:NDC * HPD // 2], wv[:, 0:NDC * HPD // 2])
        nc.sync.dma_start(wv_t[:, NDC * HPD // 2:], wv[:, NDC * HPD // 2:])
        avs0 = attention_pack(0, 0, with_kv_proj=True)

        # wo arrives while attention runs; needed first at proj_out(0)
        wo_t = cpool.tile([128, NPACK * D], F16)
        nc.sync.dma_start(wo_t[:], wo[:])

        ohs = []
        normalize_pack(avs0, 0, ohs)
        tasks1 = {2: [lambda: qk_proj(qp, wq_t, qh, CST_BQ, 1)]}
        avs1 = attention_pack(0, 1, tasks=tasks1)
        normalize_pack(avs1, 1, ohs)

        prev_ohs = ohs
        for qc in range(1, NSC):
            # while this qc's (ScalarE-paced) sweeps run, fill the PE's slack
            # with the previous qc's out-projection (4 groups per pack sweep),
            # the next qc's q-projection chunk, and (last qc) pack0's half of
            # this qc's out-projection
            tasks0, tasks1 = {}, {}
            po, pq = prev_ohs, qc - 1
            for g in range(4):
                tasks0.setdefault(2 * g + 1, []).append(
                    (lambda s=g // 2, d=g % 2, o=po, q=pq:
                     proj_out_group(q, o, s, d)))
            for g in range(4, 8):
                tasks1.setdefault(2 * (g - 4) + 1, []).append(
                    (lambda s=g // 2, d=g % 2, o=po, q=pq:
                     proj_out_group(q, o, s, d)))
            if qc < NSC - 1:
                tasks1.setdefault(0, []).append(
                    lambda s=qc + 1: qk_proj(qp, wq_t, qh, CST_BQ, s))
            ohs = []
            avs0 = attention_pack(qc, 0, tasks=tasks0)
            normalize_pack(avs0, 0, ohs, low_latency=(qc == NSC - 1))
            if qc == NSC - 1:
                # pack0's half of the last out-projection runs inside pack1's
                # final sweep, into the out2 slab (host adds it back)
                oh3 = ohs
                for g in range(8):
                    tasks1.setdefault(g, []).append(
                        (lambda s=g // 2, d=g % 2:
                         proj_out_group(0, oh3, s, d, prs=(0,),
                                        dest=out2_d, row_base=0)))
            avs1 = attention_pack(qc, 1, tasks=tasks1)
            normalize_pack(avs1, 1, ohs, low_latency=(qc == NSC - 1))
            prev_ohs = ohs
        # tail: only pack1's half of the last q-chunk remains
        for sbl in range(4):
            for dc2 in range(2):
                proj_out_group(NSC - 1, ohs, sbl, dc2, prs=(1,))

    nc.compile()
    return nc


_programs = {}


def _get_program(causal: bool):
    if causal not in _programs:
        _programs[causal] = _build(causal)
    return _programs[causal]


def _make_cst(bq4, bk4, bv4):
    """Per-core fp32 constant blob [128, CST_COLS]."""
    cst = np.zeros((128, CST_COLS), np.float32)
    # per-pack per-partition biases: partition p of pack pk is d = pk*128+p
    cst[:, CST_BQ:CST_BQ + 2] = bq4.reshape(2, 128).T
    cst[:, CST_BK:CST_BK + 2] = bk4.reshape(2, 128).T
    # bv in free-dim layout [4*64], broadcast along partitions
    cst[:, CST_BV:CST_BV + HPD] = np.broadcast_to(bv4, (128, HPD))
    cst[:, CST_LOG2] = LOG2
    cst[:, CST_ZERO] = 0.0
    cst[:, CST_ONE] = 1.0
    return cst


def _make_mask(causal: bool) -> np.ndarray:
    """Diagonal-block additive log-masks [128, 4*SC]: log(2) iff
    q_local - 128*delta >= k_local (else 0); zeros when not causal."""
    m = np.zeros((128, 4 * SC), np.float32)
    if causal:
        kloc = np.arange(128)[:, None]
        qloc = np.arange(SC)[None, :]
        for delta in range(4):
            m[:, delta * SC:(delta + 1) * SC] = np.where(
                qloc - 128 * delta >= kloc, LOG2, 0.0)
    return m.astype(np.float16)


def _pack_xT(x):
    """[S, D] -> flat [128, NSC*NDC*SC] fp16: col ((sc*NDC)+c)*SC + s holds
    x[sc*SC+s, c*128+p]."""
    xT = np.ascontiguousarray(x.T, np.float16)          # [D, S]
    return np.ascontiguousarray(
        xT.reshape(NDC, 128, NSC, SC).transpose(1, 2, 0, 3).reshape(
            128, NSC * NDC * SC))


def _pack_vT(x):
    """[S, D] -> flat [128, NKB*NDC*128] fp16: col ((sb*NDC)+c)*128 + j holds
    x[sb*128+j, c*128+p]."""
    xT = np.ascontiguousarray(x.T, np.float16)          # [D, S]
    return np.ascontiguousarray(
        xT.reshape(NDC, 128, NKB, 128).transpose(1, 2, 0, 3).reshape(
            128, NKB * NDC * 128))


def _pack_w(w):
    """[D, HPD] -> flat [128, NDC*HPD] fp16: col c*HPD + m holds w[c*128+p, m]."""
    w16 = np.asarray(w, np.float16)
    return np.ascontiguousarray(
        w16.reshape(NDC, 128, HPD).transpose(1, 0, 2).reshape(128, NDC * HPD))


def _pack_wo(w):
    """[HPD, D] -> flat [128, NPACK*D] fp16: col r*D + n holds w[r*128+p, n]."""
    w16 = np.asarray(w, np.float16)
    return np.ascontiguousarray(
        w16.reshape(NPACK, 128, D).transpose(1, 0, 2).reshape(128, NPACK * D))


def kernel(**inputs) -> np.ndarray:
    q = np.asarray(inputs["q"], np.float32)
    k = np.asarray(inputs["k"], np.float32)
    v = np.asarray(inputs["v"], np.float32)
    Wq = np.asarray(inputs["Wq"], np.float32)
    Wk = np.asarray(inputs["Wk"], np.float32)
    Wv = np.asarray(inputs["Wv"], np.float32)
    Wo = np.asarray(inputs["Wo"], np.float32)
    bq = np.asarray(inputs["bq"], np.float32)
    bk = np.asarray(inputs["bk"], np.float32)
    bv = np.asarray(inputs["bv"], np.float32)
    bo = np.asarray(inputs["bo"], np.float32)
    causal = bool(np.asarray(inputs["use_causal_mask"]).item())

    nc = _get_program(causal)

    qpb = [_pack_xT(q[b]) for b in range(B)]
    kpb = [_pack_xT(k[b]) for b in range(B)]
    vpb = [_pack_vT(v[b]) for b in range(B)]
    mask = _make_mask(causal)

    in_maps = []
    for c in range(NCORES):
        b, hg = divmod(c, NCORES // B)
        cols = slice(hg * HPD, (hg + 1) * HPD)
        in_maps.append({
            "qp": qpb[b],
            "kp": kpb[b],
            "vp": vpb[b],
            "wq": _pack_w(Wq[:, cols]),
            "wk": _pack_w(Wk[:, cols]),
            "wv": _pack_w(Wv[:, cols]),
            "wo": _pack_wo(Wo[cols, :]),
            "cst": _make_cst(bq[cols], bk[cols], bv[cols]),
            "msk": mask,
        })

    res = run_bass_kernel_spmd(nc, in_maps, list(range(NCORES)))

    out = np.empty((B, S, D), np.float32)
    ncb = NCORES // B
    for b in range(B):
        acc = res.results[b * ncb]["out"].astype(np.float32)
        acc[(NSC - 1) * SC:] += res.results[b * ncb]["out2"].astype(np.float32)
        for c in range(b * ncb + 1, (b + 1) * ncb):
            acc += res.results[c]["out"].astype(np.float32)
            acc[(NSC - 1) * SC:] += res.results[c]["out2"].astype(np.float32)
        out[b] = acc + bo
    return out
